# revision 23
# baseline (speedup 1.0000x reference)
"""KGNN head kernel for Trainium2 (Bass/Tile), 8-core data-parallel SPMD.

Computation (per batch b):
    score[g, n] = sum_d drug[b, g, d] * rel[b, 8g+n, d]         (n in 0..8)
    out[b, g, d] = sum_n score[g, n] * ent[b, 8g+n, d]

Layout: partition p = (q t) where q indexes the 16 batches of a superblock
and t the 8 group-blocks (8 groups each) of that batch. Each partition holds
64 consecutive neighbor slots x 64 dims = 16 KiB of contiguous HBM for
rel/ent, so the 16 DMA engines stream near their ~24 GB/s each.

Compute per superblock:
  - DVE tensor_tensor: prod = rel * broadcast(drug)   -> bf16
  - DVE tensor_reduce: score = sum_d prod             (bf16 in, f32 out)
  - GpSimd tensor_tensor: w = ent * broadcast(score)  -> bf16, written
    neighbor-major so each matmul rhs slice is contiguous
  - 8 PSUM-accumulating bf16 matmuls with identity lhsT sum over neighbors
  - ACT copy psum -> out tile, DMA out

The two big input streams are issued on different HWDGE rings (rel on SP,
ent on ACT). The last superblock's compute is split into halves so the
end-of-kernel serial chain (which runs after the final DMA) is shorter.
"""

import numpy as np

import concourse.bass as bass  # noqa: F401  (engine namespaces via nc)
import concourse.mybir as mybir
import concourse.tile as tile
from concourse import bacc
from concourse.bass_utils import run_bass_kernel_spmd
from concourse.masks import make_identity

F32 = mybir.dt.float32
BF16 = mybir.dt.bfloat16

N_CORES = 8
B_FULL = 2048
B_LOCAL = B_FULL // N_CORES  # 256
G = 64          # groups per sample
NN = 8          # neighbors per group
D = 64          # feature dim
S = G * NN      # 512 neighbor slots

Q = 16          # batches per superblock (partition-major)
T = 8           # group-blocks per batch (partition-minor); T*GG = G
GG = 8          # groups per group-block

H_LAST = 2      # compute chunks for the final superblock (tail latency cut)


def _build_nc(b_local: int = B_LOCAL) -> "bacc.Bacc":
    n_sblk = b_local // Q
    assert n_sblk * Q == b_local

    nc = bacc.Bacc("TRN2", target_bir_lowering=False, debug=False)

    drug_d = nc.dram_tensor("drug", [b_local, G, D], F32, kind="ExternalInput")
    rel_d = nc.dram_tensor("rel", [b_local, S, D], F32, kind="ExternalInput")
    ent_d = nc.dram_tensor("ent", [b_local, S, D], F32, kind="ExternalInput")
    out_d = nc.dram_tensor("out", [b_local, G, D], F32, kind="ExternalOutput")

    # partition p = (q t); rel/ent free = (gg n d) -> 16KB contiguous HBM runs
    rel_v = rel_d[:].rearrange(
        "(s q) (t gg n) d -> s (q t) (gg n d)", q=Q, t=T, gg=GG, n=NN
    )
    ent_v = ent_d[:].rearrange(
        "(s q) (t gg n) d -> s (q t) (gg n d)", q=Q, t=T, gg=GG, n=NN
    )
    drug_v = drug_d[:].rearrange("(s q) (t gg) d -> s (q t) (gg d)", q=Q, t=T, gg=GG)
    out_v = out_d[:].rearrange("(s q) (t gg) d -> s (q t) gg d", q=Q, t=T, gg=GG)

    with tile.TileContext(nc) as tc:
        with (
            tc.tile_pool(name="const", bufs=1) as const_pool,
            tc.tile_pool(name="rel", bufs=2) as rel_pool,
            tc.tile_pool(name="ent", bufs=2) as ent_pool,
            tc.tile_pool(name="drug", bufs=2) as drug_pool,
            tc.tile_pool(name="prod", bufs=4) as prod_pool,
            tc.tile_pool(name="score", bufs=4) as score_pool,
            tc.tile_pool(name="w", bufs=4) as w_pool,
            tc.tile_pool(name="outs", bufs=4) as out_pool,
            tc.tile_pool(name="psum", bufs=4, space="PSUM") as psum_pool,
        ):
            ident = const_pool.tile([128, 128], BF16)
            make_identity(nc, ident[:])

            def compute_chunk(sb, rel_t, ent_t, drug_t, g0, ng):
                """score+weighted-sum for groups [g0, g0+ng) of superblock sb."""
                ro = g0 * NN * D
                do = g0 * D

                prod_t = prod_pool.tile([128, ng * NN * D], BF16)
                nc.vector.tensor_tensor(
                    out=prod_t[:].rearrange("p (gg n d) -> p gg n d", gg=ng, n=NN),
                    in0=rel_t[:, ro : ro + ng * NN * D].rearrange(
                        "p (gg n d) -> p gg n d", gg=ng, n=NN
                    ),
                    in1=drug_t[:, do : do + ng * D]
                    .rearrange("p (gg n d) -> p gg n d", gg=ng, n=1)
                    .to_broadcast([128, ng, NN, D]),
                    op=mybir.AluOpType.mult,
                )
                score_t = score_pool.tile([128, ng * NN], F32)
                nc.vector.tensor_reduce(
                    out=score_t[:],
                    in_=prod_t[:].rearrange("p (gn d) -> p gn d", d=D),
                    axis=mybir.AxisListType.X,
                    op=mybir.AluOpType.add,
                )

                w_t = w_pool.tile([128, ng * NN * D], BF16)
                nc.gpsimd.tensor_tensor(
                    out=w_t[:].rearrange("p (n gg d) -> p gg n d", n=NN, gg=ng),
                    in0=ent_t[:, ro : ro + ng * NN * D].rearrange(
                        "p (gg n d) -> p gg n d", gg=ng, n=NN
                    ),
                    in1=score_t[:]
                    .rearrange("p (gg n o) -> p gg n o", gg=ng, o=1)
                    .to_broadcast([128, ng, NN, D]),
                    op=mybir.AluOpType.mult,
                )

                psum_t = psum_pool.tile([128, ng * D], F32)
                for c in range(NN):
                    nc.tensor.matmul(
                        out=psum_t[:],
                        lhsT=ident[:],
                        rhs=w_t[:, c * ng * D : (c + 1) * ng * D],
                        start=(c == 0),
                        stop=(c == NN - 1),
                    )

                out_t = out_pool.tile([128, ng * D], F32)
                nc.scalar.copy(out=out_t[:], in_=psum_t[:])
                nc.gpsimd.dma_start(out=out_v[sb][:, g0 : g0 + ng], in_=out_t[:])

            for sb in range(n_sblk):
                rel_t = rel_pool.tile([128, GG * NN * D], F32)
                nc.sync.dma_start(out=rel_t[:], in_=rel_v[sb])
                ent_t = ent_pool.tile([128, GG * NN * D], F32)
                nc.scalar.dma_start(out=ent_t[:], in_=ent_v[sb])
                drug_t = drug_pool.tile([128, GG * D], F32)
                nc.gpsimd.dma_start(out=drug_t[:], in_=drug_v[sb])

                # half-granularity compute everywhere: score_a is ready after
                # half the DVE work, so the GpSimd/PE stages start earlier
                # (compute pools are bufs=4 = 2 halves x 2 superblocks deep)
                gh = GG // 2
                for h in range(2):
                    compute_chunk(sb, rel_t, ent_t, drug_t, h * gh, gh)

    nc.compile()
    return nc


_NC_CACHE: dict = {}


def _get_nc(b_local: int = B_LOCAL):
    if b_local not in _NC_CACHE:
        _NC_CACHE[b_local] = _build_nc(b_local)
    return _NC_CACHE[b_local]


def run_sharded(drug, rel, ent, trace: bool = False):
    """Shard batch dim across the 8 cores, run, gather. Returns
    (full output [B, G, D], BassKernelResults)."""
    drug = np.ascontiguousarray(np.asarray(drug, dtype=np.float32))
    rel = np.ascontiguousarray(np.asarray(rel, dtype=np.float32))
    ent = np.ascontiguousarray(np.asarray(ent, dtype=np.float32))
    b = drug.shape[0]
    nb = b // N_CORES
    assert nb * N_CORES == b
    nc = _get_nc(nb)
    in_maps = [
        {
            "drug": np.ascontiguousarray(drug[i * nb : (i + 1) * nb]),
            "rel": np.ascontiguousarray(rel[i * nb : (i + 1) * nb]),
            "ent": np.ascontiguousarray(ent[i * nb : (i + 1) * nb]),
        }
        for i in range(N_CORES)
    ]
    last_exc = None
    for attempt in range(3):
        try:
            res = run_bass_kernel_spmd(nc, in_maps, list(range(N_CORES)), trace=trace)
            break
        except Exception as exc:  # transient device-unrecoverable states
            last_exc = exc
            import time

            time.sleep(10 * (attempt + 1))
    else:
        raise last_exc
    out = np.concatenate([res.results[i]["out"] for i in range(N_CORES)], axis=0)
    return out, res


def kernel(drug, rel, ent):
    out, _ = run_sharded(drug, rel, ent, trace=False)
    return out


# revision 27
# speedup vs baseline: 1.0179x; 1.0179x over previous
"""KGNN head kernel for Trainium2 (Bass/Tile), 8-core data-parallel SPMD.

Computation (per batch b):
    score[g, n] = sum_d drug[b, g, d] * rel[b, 8g+n, d]         (n in 0..8)
    out[b, g, d] = sum_n score[g, n] * ent[b, 8g+n, d]

Layout: partition p = (q t) where q indexes the 16 batches of a superblock
and t the 8 group-blocks (8 groups each) of that batch. Each partition holds
64 consecutive neighbor slots x 64 dims = 16 KiB of contiguous HBM for
rel/ent, so the 16 DMA engines stream near their ~24 GB/s each.

Compute per superblock:
  - DVE tensor_tensor: prod = rel * broadcast(drug)   -> bf16
  - DVE tensor_reduce: score = sum_d prod             (bf16 in, f32 out)
  - GpSimd tensor_tensor: w = ent * broadcast(score)  -> bf16, written
    neighbor-major so each matmul rhs slice is contiguous
  - 8 PSUM-accumulating bf16 matmuls with identity lhsT sum over neighbors
  - ACT copy psum -> out tile, DMA out

The two big input streams are issued on different HWDGE rings (rel on SP,
ent on ACT). The last superblock's compute is split into halves so the
end-of-kernel serial chain (which runs after the final DMA) is shorter.
"""

import numpy as np

import concourse.bass as bass  # noqa: F401  (engine namespaces via nc)
import concourse.mybir as mybir
import concourse.tile as tile
from concourse import bacc
from concourse.bass_utils import run_bass_kernel_spmd
from concourse.masks import make_identity

F32 = mybir.dt.float32
BF16 = mybir.dt.bfloat16

N_CORES = 8
B_FULL = 2048
B_LOCAL = B_FULL // N_CORES  # 256
G = 64          # groups per sample
NN = 8          # neighbors per group
D = 64          # feature dim
S = G * NN      # 512 neighbor slots

Q = 16          # batches per superblock (partition-major)
T = 8           # group-blocks per batch (partition-minor); T*GG = G
GG = 8          # groups per group-block

H_LAST = 2      # compute chunks for the final superblock (tail latency cut)


def _build_nc(b_local: int = B_LOCAL) -> "bacc.Bacc":
    n_sblk = b_local // Q
    assert n_sblk * Q == b_local

    nc = bacc.Bacc("TRN2", target_bir_lowering=False, debug=False)

    drug_d = nc.dram_tensor("drug", [b_local, G, D], F32, kind="ExternalInput")
    rel_d = nc.dram_tensor("rel", [b_local, S, D], F32, kind="ExternalInput")
    ent_d = nc.dram_tensor("ent", [b_local, S, D], F32, kind="ExternalInput")
    out_d = nc.dram_tensor("out", [b_local, G, D], F32, kind="ExternalOutput")

    # partition p = (q t); rel/ent free = (gg n d) -> 16KB contiguous HBM runs
    rel_v = rel_d[:].rearrange(
        "(s q) (t gg n) d -> s (q t) (gg n d)", q=Q, t=T, gg=GG, n=NN
    )
    ent_v = ent_d[:].rearrange(
        "(s q) (t gg n) d -> s (q t) (gg n d)", q=Q, t=T, gg=GG, n=NN
    )
    drug_v = drug_d[:].rearrange("(s q) (t gg) d -> s (q t) (gg d)", q=Q, t=T, gg=GG)
    out_v = out_d[:].rearrange("(s q) (t gg) d -> s (q t) gg d", q=Q, t=T, gg=GG)

    with tile.TileContext(nc) as tc:
        with (
            tc.tile_pool(name="const", bufs=1) as const_pool,
            tc.tile_pool(name="rel", bufs=2) as rel_pool,
            tc.tile_pool(name="ent", bufs=2) as ent_pool,
            tc.tile_pool(name="drug", bufs=2) as drug_pool,
            tc.tile_pool(name="prod", bufs=2) as prod_pool,
            tc.tile_pool(name="score", bufs=2) as score_pool,
            tc.tile_pool(name="w", bufs=2) as w_pool,
            tc.tile_pool(name="outs", bufs=2) as out_pool,
            tc.tile_pool(name="psum", bufs=2, space="PSUM") as psum_pool,
        ):
            ident = const_pool.tile([128, 128], BF16)
            make_identity(nc, ident[:])

            def compute_chunk(sb, rel_t, ent_t, drug_t, g0, ng, w_eng=None):
                """score+weighted-sum for groups [g0, g0+ng) of superblock sb."""
                ro = g0 * NN * D
                do = g0 * D

                prod_t = prod_pool.tile([128, ng * NN * D], BF16)
                nc.vector.tensor_tensor(
                    out=prod_t[:].rearrange("p (gg n d) -> p gg n d", gg=ng, n=NN),
                    in0=rel_t[:, ro : ro + ng * NN * D].rearrange(
                        "p (gg n d) -> p gg n d", gg=ng, n=NN
                    ),
                    in1=drug_t[:, do : do + ng * D]
                    .rearrange("p (gg n d) -> p gg n d", gg=ng, n=1)
                    .to_broadcast([128, ng, NN, D]),
                    op=mybir.AluOpType.mult,
                )
                score_t = score_pool.tile([128, ng * NN], F32)
                nc.vector.tensor_reduce(
                    out=score_t[:],
                    in_=prod_t[:].rearrange("p (gn d) -> p gn d", d=D),
                    axis=mybir.AxisListType.X,
                    op=mybir.AluOpType.add,
                )

                w_t = w_pool.tile([128, ng * NN * D], BF16)
                (w_eng or nc.gpsimd).tensor_tensor(
                    out=w_t[:].rearrange("p (n gg d) -> p gg n d", n=NN, gg=ng),
                    in0=ent_t[:, ro : ro + ng * NN * D].rearrange(
                        "p (gg n d) -> p gg n d", gg=ng, n=NN
                    ),
                    in1=score_t[:]
                    .rearrange("p (gg n o) -> p gg n o", gg=ng, o=1)
                    .to_broadcast([128, ng, NN, D]),
                    op=mybir.AluOpType.mult,
                )

                psum_t = psum_pool.tile([128, ng * D], F32)
                for c in range(NN):
                    nc.tensor.matmul(
                        out=psum_t[:],
                        lhsT=ident[:],
                        rhs=w_t[:, c * ng * D : (c + 1) * ng * D],
                        start=(c == 0),
                        stop=(c == NN - 1),
                    )

                out_t = out_pool.tile([128, ng * D], F32)
                nc.scalar.copy(out=out_t[:], in_=psum_t[:])
                nc.gpsimd.dma_start(out=out_v[sb][:, g0 : g0 + ng], in_=out_t[:])

            for sb in range(n_sblk):
                rel_t = rel_pool.tile([128, GG * NN * D], F32)
                nc.sync.dma_start(out=rel_t[:], in_=rel_v[sb])
                ent_t = ent_pool.tile([128, GG * NN * D], F32)
                nc.scalar.dma_start(out=ent_t[:], in_=ent_v[sb])
                drug_t = drug_pool.tile([128, GG * D], F32)
                nc.gpsimd.dma_start(out=drug_t[:], in_=drug_v[sb])

                if sb == n_sblk - 1 and H_LAST > 1:
                    # drain phase: DVE is idle after the last reduce while the
                    # GpSimd queue is still draining the previous w -- run the
                    # final superblock's w on DVE to shorten the tail chain
                    gh = GG // H_LAST
                    for h in range(H_LAST):
                        compute_chunk(
                            sb, rel_t, ent_t, drug_t, h * gh, gh, w_eng=nc.vector
                        )
                else:
                    compute_chunk(sb, rel_t, ent_t, drug_t, 0, GG)

    nc.compile()
    return nc


_NC_CACHE: dict = {}


def _get_nc(b_local: int = B_LOCAL):
    if b_local not in _NC_CACHE:
        _NC_CACHE[b_local] = _build_nc(b_local)
    return _NC_CACHE[b_local]


def run_sharded(drug, rel, ent, trace: bool = False):
    """Shard batch dim across the 8 cores, run, gather. Returns
    (full output [B, G, D], BassKernelResults)."""
    drug = np.ascontiguousarray(np.asarray(drug, dtype=np.float32))
    rel = np.ascontiguousarray(np.asarray(rel, dtype=np.float32))
    ent = np.ascontiguousarray(np.asarray(ent, dtype=np.float32))
    b = drug.shape[0]
    nb = b // N_CORES
    assert nb * N_CORES == b
    nc = _get_nc(nb)
    in_maps = [
        {
            "drug": np.ascontiguousarray(drug[i * nb : (i + 1) * nb]),
            "rel": np.ascontiguousarray(rel[i * nb : (i + 1) * nb]),
            "ent": np.ascontiguousarray(ent[i * nb : (i + 1) * nb]),
        }
        for i in range(N_CORES)
    ]
    last_exc = None
    for attempt in range(3):
        try:
            res = run_bass_kernel_spmd(nc, in_maps, list(range(N_CORES)), trace=trace)
            break
        except Exception as exc:  # transient device-unrecoverable states
            last_exc = exc
            import time

            time.sleep(10 * (attempt + 1))
    else:
        raise last_exc
    out = np.concatenate([res.results[i]["out"] for i in range(N_CORES)], axis=0)
    return out, res


def kernel(drug, rel, ent):
    out, _ = run_sharded(drug, rel, ent, trace=False)
    return out


# revision 28
# speedup vs baseline: 1.0265x; 1.0085x over previous
"""KGNN head kernel for Trainium2 (Bass/Tile), 8-core data-parallel SPMD.

Computation (per batch b):
    score[g, n] = sum_d drug[b, g, d] * rel[b, 8g+n, d]         (n in 0..8)
    out[b, g, d] = sum_n score[g, n] * ent[b, 8g+n, d]

Layout: partition p = (q t) where q indexes the 16 batches of a superblock
and t the 8 group-blocks (8 groups each) of that batch. Each partition holds
64 consecutive neighbor slots x 64 dims = 16 KiB of contiguous HBM for
rel/ent, so the 16 DMA engines stream near their ~24 GB/s each.

Compute per superblock:
  - DVE tensor_tensor: prod = rel * broadcast(drug)   -> bf16
  - DVE tensor_reduce: score = sum_d prod             (bf16 in, f32 out)
  - GpSimd tensor_tensor: w = ent * broadcast(score)  -> bf16, written
    neighbor-major so each matmul rhs slice is contiguous
  - 8 PSUM-accumulating bf16 matmuls with identity lhsT sum over neighbors
  - ACT copy psum -> out tile, DMA out

The two big input streams are issued on different HWDGE rings (rel on SP,
ent on ACT). The last superblock's compute is split into halves so the
end-of-kernel serial chain (which runs after the final DMA) is shorter.
"""

import numpy as np

import concourse.bass as bass  # noqa: F401  (engine namespaces via nc)
import concourse.mybir as mybir
import concourse.tile as tile
from concourse import bacc
from concourse.bass_utils import run_bass_kernel_spmd
from concourse.masks import make_identity

F32 = mybir.dt.float32
BF16 = mybir.dt.bfloat16

N_CORES = 8
B_FULL = 2048
B_LOCAL = B_FULL // N_CORES  # 256
G = 64          # groups per sample
NN = 8          # neighbors per group
D = 64          # feature dim
S = G * NN      # 512 neighbor slots

Q = 16          # batches per superblock (partition-major)
T = 8           # group-blocks per batch (partition-minor); T*GG = G
GG = 8          # groups per group-block

H_LAST = 2      # compute chunks for the final superblock (tail latency cut)


def _build_nc(b_local: int = B_LOCAL) -> "bacc.Bacc":
    n_sblk = b_local // Q
    assert n_sblk * Q == b_local

    nc = bacc.Bacc("TRN2", target_bir_lowering=False, debug=False)

    drug_d = nc.dram_tensor("drug", [b_local, G, D], F32, kind="ExternalInput")
    rel_d = nc.dram_tensor("rel", [b_local, S, D], F32, kind="ExternalInput")
    ent_d = nc.dram_tensor("ent", [b_local, S, D], F32, kind="ExternalInput")
    out_d = nc.dram_tensor("out", [b_local, G, D], F32, kind="ExternalOutput")

    # partition p = (q t); rel/ent free = (gg n d) -> 16KB contiguous HBM runs
    rel_v = rel_d[:].rearrange(
        "(s q) (t gg n) d -> s (q t) (gg n d)", q=Q, t=T, gg=GG, n=NN
    )
    ent_v = ent_d[:].rearrange(
        "(s q) (t gg n) d -> s (q t) (gg n d)", q=Q, t=T, gg=GG, n=NN
    )
    drug_v = drug_d[:].rearrange("(s q) (t gg) d -> s (q t) (gg d)", q=Q, t=T, gg=GG)
    out_v = out_d[:].rearrange("(s q) (t gg) d -> s (q t) gg d", q=Q, t=T, gg=GG)

    with tile.TileContext(nc) as tc:
        with (
            tc.tile_pool(name="const", bufs=1) as const_pool,
            tc.tile_pool(name="rel", bufs=2) as rel_pool,
            tc.tile_pool(name="ent", bufs=2) as ent_pool,
            tc.tile_pool(name="drug", bufs=2) as drug_pool,
            tc.tile_pool(name="prod", bufs=2) as prod_pool,
            tc.tile_pool(name="score", bufs=2) as score_pool,
            tc.tile_pool(name="w", bufs=2) as w_pool,
            tc.tile_pool(name="outs", bufs=2) as out_pool,
            tc.tile_pool(name="psum", bufs=2, space="PSUM") as psum_pool,
        ):
            ident = const_pool.tile([128, 128], BF16)
            make_identity(nc, ident[:])

            def compute_chunk(sb, rel_t, ent_t, drug_t, g0, ng, w_eng=None):
                """score+weighted-sum for groups [g0, g0+ng) of superblock sb."""
                ro = g0 * NN * D
                do = g0 * D

                prod_t = prod_pool.tile([128, ng * NN * D], BF16)
                nc.vector.tensor_tensor(
                    out=prod_t[:].rearrange("p (gg n d) -> p gg n d", gg=ng, n=NN),
                    in0=rel_t[:, ro : ro + ng * NN * D].rearrange(
                        "p (gg n d) -> p gg n d", gg=ng, n=NN
                    ),
                    in1=drug_t[:, do : do + ng * D]
                    .rearrange("p (gg n d) -> p gg n d", gg=ng, n=1)
                    .to_broadcast([128, ng, NN, D]),
                    op=mybir.AluOpType.mult,
                )
                score_t = score_pool.tile([128, ng * NN], F32)
                nc.vector.tensor_reduce(
                    out=score_t[:],
                    in_=prod_t[:].rearrange("p (gn d) -> p gn d", d=D),
                    axis=mybir.AxisListType.X,
                    op=mybir.AluOpType.add,
                )

                w_t = w_pool.tile([128, ng * NN * D], BF16)
                (w_eng or nc.gpsimd).tensor_tensor(
                    out=w_t[:].rearrange("p (n gg d) -> p gg n d", n=NN, gg=ng),
                    in0=ent_t[:, ro : ro + ng * NN * D].rearrange(
                        "p (gg n d) -> p gg n d", gg=ng, n=NN
                    ),
                    in1=score_t[:]
                    .rearrange("p (gg n o) -> p gg n o", gg=ng, o=1)
                    .to_broadcast([128, ng, NN, D]),
                    op=mybir.AluOpType.mult,
                )

                psum_t = psum_pool.tile([128, ng * D], F32)
                for c in range(NN):
                    nc.tensor.matmul(
                        out=psum_t[:],
                        lhsT=ident[:],
                        rhs=w_t[:, c * ng * D : (c + 1) * ng * D],
                        start=(c == 0),
                        stop=(c == NN - 1),
                    )

                out_t = out_pool.tile([128, ng * D], F32)
                nc.scalar.copy(out=out_t[:], in_=psum_t[:])
                nc.gpsimd.dma_start(out=out_v[sb][:, g0 : g0 + ng], in_=out_t[:])

            for sb in range(n_sblk):
                rel_t = rel_pool.tile([128, GG * NN * D], F32)
                nc.sync.dma_start(out=rel_t[:], in_=rel_v[sb])
                ent_t = ent_pool.tile([128, GG * NN * D], F32)
                nc.scalar.dma_start(out=ent_t[:], in_=ent_v[sb])
                drug_t = drug_pool.tile([128, GG * D], F32)
                nc.gpsimd.dma_start(out=drug_t[:], in_=drug_v[sb])

                # drain phase: DVE idles at the tail while the GpSimd queue
                # drains its last w's -- run the final two superblocks' w on
                # DVE to shorten the end-of-kernel chain
                if sb == n_sblk - 1 and H_LAST > 1:
                    gh = GG // H_LAST
                    for h in range(H_LAST):
                        compute_chunk(
                            sb, rel_t, ent_t, drug_t, h * gh, gh, w_eng=nc.vector
                        )
                elif sb == n_sblk - 2:
                    compute_chunk(sb, rel_t, ent_t, drug_t, 0, GG, w_eng=nc.vector)
                else:
                    compute_chunk(sb, rel_t, ent_t, drug_t, 0, GG)

    nc.compile()
    return nc


_NC_CACHE: dict = {}


def _get_nc(b_local: int = B_LOCAL):
    if b_local not in _NC_CACHE:
        _NC_CACHE[b_local] = _build_nc(b_local)
    return _NC_CACHE[b_local]


def run_sharded(drug, rel, ent, trace: bool = False):
    """Shard batch dim across the 8 cores, run, gather. Returns
    (full output [B, G, D], BassKernelResults)."""
    drug = np.ascontiguousarray(np.asarray(drug, dtype=np.float32))
    rel = np.ascontiguousarray(np.asarray(rel, dtype=np.float32))
    ent = np.ascontiguousarray(np.asarray(ent, dtype=np.float32))
    b = drug.shape[0]
    nb = b // N_CORES
    assert nb * N_CORES == b
    nc = _get_nc(nb)
    in_maps = [
        {
            "drug": np.ascontiguousarray(drug[i * nb : (i + 1) * nb]),
            "rel": np.ascontiguousarray(rel[i * nb : (i + 1) * nb]),
            "ent": np.ascontiguousarray(ent[i * nb : (i + 1) * nb]),
        }
        for i in range(N_CORES)
    ]
    last_exc = None
    for attempt in range(3):
        try:
            res = run_bass_kernel_spmd(nc, in_maps, list(range(N_CORES)), trace=trace)
            break
        except Exception as exc:  # transient device-unrecoverable states
            last_exc = exc
            import time

            time.sleep(10 * (attempt + 1))
    else:
        raise last_exc
    out = np.concatenate([res.results[i]["out"] for i in range(N_CORES)], axis=0)
    return out, res


def kernel(drug, rel, ent):
    out, _ = run_sharded(drug, rel, ent, trace=False)
    return out
